# revision 1
# baseline (speedup 1.0000x reference)
"""ConvNeXt composite loss (attention-BCE + dice + reverse-dice) on 8 trn2 cores.

Data-parallel: batch dim 16 -> 2 per core. Each core reduces its shard to a
small vector of partial sums; the host assembles the final scalar in float64.

Math (labels t are exactly {0,1}, IOU coeff is 0):
  q = |p - t|   ->  weight w = 8^sqrt(q),  log-term L = ln(1-q)
  per-batch   S1 = sum(t * w * L), Sz = sum(w * L), S0 = Sz - S1
  attention loss = -sum_b [alpha_b * S1_b + (1-alpha_b) * S0_b],
    alpha_b = (total - num_pos_b) / total
  per-(b,c)   St, Sp, Sp2, and Spt = (Sp + St - Sq)/2   (since
    sum|p-t| = Sp + St - 2*Spt for t in {0,1})
  dice / reverse dice follow algebraically from (St, Sp, Spt, Sp2).

Device per unit (16 half-planes of [128,1024] per core):
  DVE   : q = |p-t| (custom fused op, accum Sq); z = w*L (TTR, accum Sz)
  ACT   : sqrt(q) -> sq (fp16), ln(1-q) -> L, exp(ln8 * sq) -> w
  POOL  : S1 = sum(z*t) (STT accum), Sp2 = sum(p*p) (STT accum)
  PE    : ones-matmuls -> per-plane column sums of p and t in PSUM
The torch-style log clamp at -100 only matters for elements with p < 2^-25
and t == 1 (q saturates to 1.0 in f32); those are patched on upload and
corrected exactly on the host.
"""

import os
import sys

import numpy as np

if "/opt/trn_rl_repo" not in sys.path:
    sys.path.insert(0, "/opt/trn_rl_repo")

# ---------------------------------------------------------------- constants
B, C, H, W = 16, 4, 512, 512
N_CORES = 8
B_LOC = B // N_CORES              # 2 batches per core
NPLANE = B_LOC * C                # 8 planes of 512x512 per core
P = 128                           # partitions
PLANE_FD = (H * W) // P           # 2048
FD = 1024                         # half-plane free dim
NU = NPLANE * (PLANE_FD // FD)    # 16 units per core

LN8 = float(np.log(8.0))          # exp scale for 8^x
SMOOTH = 1e-6
TOTAL = float(C * H * W)
NPIX = float(H * W)

# acc column layout per unit: 4 cols [Sq, Sz, S1, Sp2]
ACC_COLS = 4 * NU                 # 64
# out vector layout: [0:64] partition-reduced acc, [64:72] Sp/plane,
# [72:80] St/plane, [80:88] Sp2/plane, [88:96] S1-odd/plane
OUT_W = 96

_CACHE = {}


def _register_absdiff():
    """Fused r = 1 - |a-b| with accum_out = per-partition sum(r), on DVE.
    The accumulator folds the f32 pipeline value even when out is fp16."""
    from operator import add

    import concourse.dve_ops as dve_ops
    from concourse.dve_ops import DveOp
    from concourse.dve_spec import One, Spec, Src0, Src1, lower, maxx
    from concourse.dve_uop import DveOpSpec

    name = "ONE_MINUS_ABSDIFF_ANT"
    for op in dve_ops.OPS:
        if op.name == name:
            return op

    def _ref(in0, in1, s0, s1, imm2):
        b = 1.0 - np.abs(in0.astype(np.float32) - in1.astype(np.float32))
        b = b.astype(np.float32)
        return b, b.reshape(b.shape[0], -1).sum(axis=-1, keepdims=True)

    spec = Spec(body=One - maxx(Src0 - Src1, Src1 - Src0), accum=add, reference=_ref)
    row = dve_ops._CUSTOM_DVE_ROW_BASE + len(dve_ops.OPS)
    shas = {}
    for ver in ("v3", "v4"):
        try:
            shas[ver] = DveOpSpec(
                name=name, opcode=row, uops=lower(spec, ver=ver), rd1_en=True
            ).sha(ver)
        except Exception:
            pass
    op = DveOp(name, spec, subdim=False, uops_sha=shas)
    dve_ops.OPS.append(op)
    dve_ops.CUSTOM_DVE_SPECS[name] = spec
    dve_ops._SUB_OPCODE_FOR_NAME[name] = row
    return op


def _register_mulred():
    """Fused z = a*b with accum_out = per-partition sum, on DVE.
    (Stock tensor_tensor_reduce crashes the exec unit with an fp8 in1.)"""
    from operator import add

    import concourse.dve_ops as dve_ops
    from concourse.dve_ops import DveOp
    from concourse.dve_spec import Spec, Src0, Src1, lower
    from concourse.dve_uop import DveOpSpec

    name = "MUL_RED_ANT"
    for op in dve_ops.OPS:
        if op.name == name:
            return op

    def _ref(in0, in1, s0, s1, imm2):
        b = (in0.astype(np.float32) * in1.astype(np.float32)).astype(np.float32)
        return b, b.reshape(b.shape[0], -1).sum(axis=-1, keepdims=True)

    spec = Spec(body=Src0 * Src1, accum=add, reference=_ref)
    row = dve_ops._CUSTOM_DVE_ROW_BASE + len(dve_ops.OPS)
    shas = {}
    for ver in ("v3", "v4"):
        try:
            shas[ver] = DveOpSpec(
                name=name, opcode=row, uops=lower(spec, ver=ver), rd1_en=True
            ).sha(ver)
        except Exception:
            pass
    op = DveOp(name, spec, subdim=False, uops_sha=shas)
    dve_ops.OPS.append(op)
    dve_ops.CUSTOM_DVE_SPECS[name] = spec
    dve_ops._SUB_OPCODE_FOR_NAME[name] = row
    return op


def _build_bass():
    """One core's module: inputs cls [8,128,2048] f32, lab [8,128,2048] fp8;
    output out [1, OUT_W] f32 of partial sums."""
    from contextlib import ExitStack

    import concourse.bacc as bacc
    import concourse.mybir as mybir
    from concourse.tile import TileContext, add_dep_helper

    dt = mybir.dt
    Alu = mybir.AluOpType
    Act = mybir.ActivationFunctionType

    absdiff = _register_absdiff()
    mulred = _register_mulred()

    nc = bacc.Bacc()
    cls = nc.declare_dram_parameter("cls", [NPLANE, P, PLANE_FD], dt.float32, isOutput=False)
    lab = nc.declare_dram_parameter("lab", [NPLANE, P, PLANE_FD], dt.float8e4, isOutput=False)
    out = nc.declare_dram_parameter("out", [1, OUT_W], dt.float32, isOutput=True)

    def chain(insts, reason):
        for a, b in zip(insts[1:], insts[:-1]):
            add_dep_helper(a.ins, b.ins, sync=False, reason=reason)

    with TileContext(nc) as tc, ExitStack() as ctx:
        pool = lambda name, bufs: ctx.enter_context(tc.tile_pool(name=name, bufs=bufs))
        p_pool = pool("p", 4)       # plane tiles [128,2048] f32
        t_pool = pool("t", NPLANE)  # plane tiles fp8, alive until the mask pass
        q_pool = pool("q", NU)      # r = 1-|p-t| tiles, fp16
        sq_pool = pool("sq", NU)    # fp16
        w_pool = pool("w", 4)       # fp16, consumed right after each Exp
        l_pool = pool("l", NU)      # fp16
        z_pool = pool("z", 4)       # fp16
        ztm_pool = pool("ztm", 4)   # fp16 masked z for odd units (PE rows)
        junk_pool = pool("junk", 2)
        p2_pool = pool("p2", 13)    # fp16 squares; live until their PE mms
        misc_pool = pool("misc", 1)
        psum_pool = ctx.enter_context(tc.tile_pool(name="ps", bufs=1, space="PSUM"))

        acc = misc_pool.tile([P, ACC_COLS], dt.float32)
        ones_f = misc_pool.tile([P, 1], dt.float32)
        ones_8 = misc_pool.tile([P, 1], dt.float8e4)
        ones_h = misc_pool.tile([P, 1], dt.float16)
        outsb = misc_pool.tile([1, OUT_W], dt.float32)
        nc.vector.memset(acc[:], 0.0)
        nc.vector.memset(ones_f[:], 1.0)
        nc.gpsimd.memset(ones_8[:], 1.0)
        nc.gpsimd.memset(ones_h[:], 1.0)

        RC = 64  # row-chunk width: keeps each rows tensor in one PSUM bank
        rows_p = psum_pool.tile([1, NPLANE * RC], dt.float32)
        rows_t = psum_pool.tile([1, NPLANE * RC], dt.float32)
        rows_p2 = psum_pool.tile([1, NPLANE * RC], dt.float32)
        rows_zt = psum_pool.tile([1, NPLANE * RC], dt.float32)
        accp = psum_pool.tile([1, ACC_COLS], dt.float32)

        pt = [None] * NPLANE
        tt = [None] * NPLANE
        qt = [None] * NU
        sqt = [None] * NU
        p2t = [None] * NU
        act_insts = []
        pe_p, pe_t, pe_p2 = [], [], []

        def half(tile, u):
            h = u % 2
            return tile[:, h * FD : (h + 1) * FD]

        # ---- loads (plane granularity) + q + pool squares
        for u in range(NU):
            plane = u // 2
            if u % 2 == 0:
                pt[plane] = p_pool.tile([P, PLANE_FD], dt.float32, tag="p", name=f"p{plane}")
                tt[plane] = t_pool.tile([P, PLANE_FD], dt.float8e4, tag="t", name=f"t{plane}")
                nc.sync.dma_start(out=pt[plane][:], in_=cls[plane])
                nc.sync.dma_start(out=tt[plane][:], in_=lab[plane])

            qt[u] = q_pool.tile([P, FD], dt.float16, tag="q", name=f"r{u}")
            nc.vector._custom_dve(
                absdiff,
                out=qt[u][:],
                in0=half(pt[plane], u),
                in1=half(tt[plane], u),
                accum_out=acc[:, 4 * u : 4 * u + 1],
            )
            p2t[u] = p2_pool.tile([P, FD], dt.float16, tag="p2", name=f"p2_{u}")
            nc.gpsimd.tensor_tensor(
                p2t[u][:], half(pt[plane], u), half(pt[plane], u), Alu.mult
            )
            # PE row sums: p (f32 ones) emitted now; ordering fixed by chains below
            plane_sl = slice((u // 2) * RC, (u // 2 + 1) * RC)
            first = u % 2 == 0
            for j in range(FD // RC):
                st_ = first and j == 0
                sp_ = (not first) and j == FD // RC - 1
                pe_p.append(nc.tensor.matmul(
                    rows_p[0:1, plane_sl], ones_f[:],
                    half(pt[plane], u)[:, j * RC : (j + 1) * RC],
                    start=st_, stop=sp_,
                ))
                pe_p2.append(nc.tensor.matmul(
                    rows_p2[0:1, plane_sl], ones_h[:],
                    p2t[u][:, j * RC : (j + 1) * RC],
                    start=st_, stop=sp_,
                ))
                pe_t.append(nc.tensor.matmul(
                    rows_t[0:1, plane_sl], ones_8[:],
                    half(tt[plane], u)[:, j * RC : (j + 1) * RC],
                    start=st_, stop=sp_,
                ))

        # ---- ACT phase A: all sqrts of (1 - r) (sqrt set)
        for u in range(NU):
            sqt[u] = sq_pool.tile([P, FD], dt.float16, tag="sq", name=f"sq{u}")
            act_insts.append(
                nc.scalar.activation(sqt[u][:], qt[u][:], Act.Sqrt, bias=1.0, scale=-1.0)
            )

        # ---- ACT phase B: all Ln, then all Exp (one set each at worst),
        # then z + mask per unit on DVE
        lts = [None] * NU
        wts = [None] * NU
        for u in range(NU):
            lts[u] = l_pool.tile([P, FD], dt.float16, tag="l", name=f"l{u}")
            act_insts.append(
                nc.scalar.activation(lts[u][:], qt[u][:], Act.Ln)
            )
        for u in range(NU):
            wts[u] = w_pool.tile([P, FD], dt.float16, tag="w", name=f"w{u}")
            act_insts.append(
                nc.scalar.activation(wts[u][:], sqt[u][:], Act.Exp, scale=LN8)
            )
        pe_zt = []
        for u in range(NU):
            plane = u // 2
            zt = z_pool.tile([P, FD], dt.float16, tag="z")
            nc.vector._custom_dve(
                mulred, out=zt[:], in0=wts[u][:], in1=lts[u][:],
                accum_out=acc[:, 4 * u + 1 : 4 * u + 2],
            )
            if u % 2 == 0:
                junk = junk_pool.tile([P, FD], dt.float16, tag="junk")
                nc.vector._custom_dve(
                    mulred, out=junk[:], in0=zt[:], in1=half(tt[plane], u),
                    accum_out=acc[:, 4 * u + 2 : 4 * u + 3],
                )
            else:
                ztm = ztm_pool.tile([P, FD], dt.float16, tag="ztm", name=f"ztm{u}")
                nc.gpsimd.tensor_tensor(ztm[:], zt[:], half(tt[plane], u), Alu.mult)
                plane_sl = slice(plane * RC, (plane + 1) * RC)
                for j in range(FD // RC):
                    pe_zt.append(nc.tensor.matmul(
                        rows_zt[0:1, plane_sl], ones_h[:],
                        ztm[:, j * RC : (j + 1) * RC],
                        start=j == 0, stop=j == FD // RC - 1,
                    ))

        # ---- finals
        accmm = nc.tensor.matmul(accp[0:1, :], ones_f[:], acc[:], start=True, stop=True)
        nc.vector.tensor_copy(outsb[0:1, 0:ACC_COLS], accp[0:1, :])
        for name, rows, col0 in (
            ("sp", rows_p, ACC_COLS),
            ("st", rows_t, ACC_COLS + NPLANE),
            ("sp2", rows_p2, ACC_COLS + 2 * NPLANE),
            ("s1o", rows_zt, ACC_COLS + 3 * NPLANE),
        ):
            nc.vector.tensor_reduce(
                out=outsb[0:1, col0 : col0 + NPLANE],
                in_=rows[0:1, :].rearrange("a (n k) -> a n k", k=RC),
                axis=mybir.AxisListType.X,
                op=Alu.add,
            )
        nc.sync.dma_start(out=out[0:1, :], in_=outsb[0:1, :])

        # ---- enforce engine-stream orders (same-engine, no semaphores):
        # ACT: sqrt set then ln/exp set -> 2 table loads total
        if os.environ.get("KB_NO_ACTCHAIN") != "1":
            chain(act_insts, "act set order")
        # PE: group by stationary dtype -> 4 ldweights total
        if os.environ.get("KB_NO_PECHAIN") != "1":
            chain(pe_p + pe_p2 + pe_t + pe_zt + [accmm], "pe stationary runs")

    nc.finalize()
    return nc


def _get_nc():
    if "nc" not in _CACHE:
        _CACHE["nc"] = _build_bass()
    return _CACHE["nc"]


def _host_prepare(cls_score, label):
    """Shard, convert label to fp8, patch log-clamp outliers.

    Returns (in_maps, corrections) where corrections[b] is the float64
    adjustment to add to S1_b (device computes a finite z for the patched
    element; the reference wants w * (-(-100))-style clamped terms)."""
    import ml_dtypes

    p = np.ascontiguousarray(cls_score.astype(np.float32, copy=False))
    t = label
    f8 = ml_dtypes.float8_e4m3fn if hasattr(ml_dtypes, "float8_e4m3fn") else ml_dtypes.float8_e4m3

    corrections = np.zeros(B, dtype=np.float64)
    # elements where q = |p-t| rounds to 1.0 in f32: t==1 and p < 2^-25
    bad = (t == 1) & (p < 2.0**-25)
    if bad.any():
        p = p.copy()
        idx = np.argwhere(bad)
        repl = np.float32(2.0**-24)
        for b_i, c_i, h_i, w_i in idx:
            p_orig = np.float64(cls_score[b_i, c_i, h_i, w_i])
            # reference term (f32 semantics): w = 8^sqrt(1-clip(p)), bce = -max(ln p, -100)
            p_clip = max(p_orig, 1e-14)
            w_true = 8.0 ** np.sqrt(1.0 - p_clip)
            l_true = max(np.log(p_orig) if p_orig > 0 else -np.inf, -100.0)
            z_true = w_true * l_true
            # device term with the patched value
            q_dev = np.float32(1.0) - repl
            z_dev = 8.0 ** np.float64(np.sqrt(q_dev)) * np.log1p(-np.float64(q_dev))
            corrections[b_i] += z_true - z_dev
            p[b_i, c_i, h_i, w_i] = repl

    in_maps = []
    for c_i in range(N_CORES):
        sh = slice(c_i * B_LOC, (c_i + 1) * B_LOC)
        cls_c = p[sh].reshape(NPLANE, P, PLANE_FD)
        lab_c = t[sh].astype(f8).reshape(NPLANE, P, PLANE_FD)
        in_maps.append({"cls": np.ascontiguousarray(cls_c), "lab": np.ascontiguousarray(lab_c)})
    return in_maps, corrections


def _assemble(outs, corrections):
    """outs: per-core [1, OUT_W] f32. Final scalar in float64."""
    loss = 0.0
    att = 0.0
    for c_i in range(N_CORES):
        v = outs[c_i].reshape(-1).astype(np.float64)
        acc = v[:ACC_COLS].reshape(NU, 4)      # per unit: Sq, Sz, S1, Sp2
        Sp_pl = v[ACC_COLS : ACC_COLS + NPLANE]
        St_pl = v[ACC_COLS + NPLANE : ACC_COLS + 2 * NPLANE]
        Sp2_pl = v[ACC_COLS + 2 * NPLANE : ACC_COLS + 3 * NPLANE]
        Sq_pl = NPIX - (acc[0::2, 0] + acc[1::2, 0])  # acc col0 holds sum(r)
        Spt_pl = 0.5 * (Sp_pl + St_pl - Sq_pl)

        # dice + reverse dice per plane
        inter2 = NPIX - Sp_pl - St_pl + Spt_pl
        denom2 = (NPIX - 2.0 * Sp_pl + Sp2_pl) + (NPIX - St_pl)
        dice = 1.0 - (2.0 * Spt_pl + SMOOTH) / (Sp2_pl + St_pl + SMOOTH)
        rdice = 1.0 - (2.0 * inter2 + SMOOTH) / (denom2 + SMOOTH)
        loss += 2500.0 * (dice.sum() + rdice.sum())

        # attention BCE per local batch
        S1o_pl = v[ACC_COLS + 3 * NPLANE : ACC_COLS + 4 * NPLANE]
        for bl in range(B_LOC):
            b_g = c_i * B_LOC + bl
            sl = slice(bl * 2 * C, (bl + 1) * 2 * C)  # this batch's 8 units
            S1 = acc[sl, 2].sum() + S1o_pl[bl * C : (bl + 1) * C].sum() + corrections[b_g]
            Sz = acc[sl, 1].sum() + corrections[b_g]
            S0 = Sz - S1
            num_pos = St_pl[bl * C : (bl + 1) * C].sum()
            alpha = (TOTAL - num_pos) / TOTAL
            att += -(alpha * S1 + (1.0 - alpha) * S0)
    return loss + att


def kernel(cls_score, label):
    from concourse.bass_utils import run_bass_kernel_spmd

    nc = _get_nc()
    in_maps, corrections = _host_prepare(np.asarray(cls_score), np.asarray(label))
    res = run_bass_kernel_spmd(
        nc, in_maps, list(range(N_CORES)), trace=os.environ.get("KERNEL_TRACE") == "1"
    )
    if os.environ.get("KERNEL_TRACE") == "1":
        _CACHE["last_results"] = res
    outs = [r["out"] for r in res.results]
    return np.float32(_assemble(outs, corrections))

